# revision 16
# baseline (speedup 1.0000x reference)
"""Trainium2 Bass kernel for dual cross-attention (CotSR) block.

Problem (hardcoded shapes): B=4, C=96, H=W=64 -> N=4096, C8=12, NC=96.
  For each direction d and batch b:
    q = wq @ x_q + bq            [12, N]
    k = wk @ x_kv + bk           [12, N]
    v = wv @ x_kv + bv           [96, N]
    S = q^T k  (S[n, m])         softmax over m (unscaled)
    O = sum_m softmax(S)[n, m] * v[:, m]
    out = wc @ (x_q + gamma * O) + bc

Sharding: 8 independent (direction, batch) units -> 8 NeuronCores.

Per-core device pipeline (v2 — bf16 matmuls, fused epilogue):
  - the output conv is folded into the V projection host-side:
    W2 = gamma*wc@wv, b2 = gamma*wc@bv, so the A@V matmul directly
    produces the final attention contribution (pre-normalization);
    Y0 = wc@x_q + bc is computed once in the projection phase.
  - all big matmuls run in bf16 (1 PE cycle/row vs 4 for fp32):
    projections (K=97 incl. bias-through-ones-row), scores (K=12),
    A@V (K=128 per m-block, 97 rows incl. ones row for the softmax
    denominator).
  - scores computed transposed (S^T[m, n]) so the A@V matmul needs no
    transposes: per n-chunk of 512 query rows, m-chunks of 3/2 PSUM
    banks ping-pong; exp on ScalarE (ACT) straight out of PSUM into
    bf16 SBUF tiles; A@V software-pipelined one m-chunk behind.
  - per-chunk epilogue (software-pipelined one n-chunk behind): copy
    O'[97,512] to SBUF, reciprocal of row 96 (=denominator) on DVE,
    rank-1 fp32r matmul broadcasts 1/l to [96,512], then DVE
    multiply + add Y0 and DMA out.
"""

import numpy as np

B, C, C8, NC = 4, 96, 12, 96
N = 4096  # 64*64
CP = 97  # C + ones row
NCH = 8  # n chunks of 512
NB = 512
MB = 32  # m blocks of 128 per n-chunk
MCH = 16  # m-chunks of 2 blocks each; uniform so sc tags alternate globally

_PROG = None


def _split_multi_waits(nc):
    """Walrus in this container rejects >1 sync wait per instruction.
    Split extra waits onto same-engine NOPs inserted just before."""
    import concourse.mybir as mybir

    n_split = 0
    for bb in nc.main_func.blocks:
        insts = list(bb.instructions)
        if not any(i.sync_info and len(i.sync_info.on_wait) > 1 for i in insts):
            continue
        new = []
        for inst in insts:
            si = inst.sync_info
            if si is not None and len(si.on_wait) > 1:
                waits = list(si.on_wait)
                for w in waits[:-1]:
                    n_split += 1
                    new.append(
                        mybir.InstNoOp(
                            name=f"{inst.name}-wsplit{n_split}",
                            engine=inst.engine,
                            ins=[],
                            outs=[],
                            sync_info=mybir.SyncInfo(on_wait=[w], on_update=[]),
                        )
                    )
                inst.sync_info = mybir.SyncInfo(
                    on_wait=[waits[-1]], on_update=list(si.on_update)
                )
            new.append(inst)
        bb.instructions = new
    return n_split


def _build_program():
    import concourse.bass as bass
    import concourse.mybir as mybir
    import concourse.tile as tile

    f32 = mybir.dt.float32
    f32r = mybir.dt.float32r
    bf16 = mybir.dt.bfloat16
    nc = bass.Bass()

    # all weights packed in one tensor: tiny-row DMAs (24B/partition) cost
    # 20-30us each on this part; one 432B-row DMA costs ~1us
    WALL = 2 * C8 + 2 * NC  # 216 cols: [wqT | wkT | wvT | wcA]
    wall_d = nc.dram_tensor("wall", [CP, WALL], bf16, kind="ExternalInput")
    xq_d = nc.dram_tensor("xq", [CP, N], bf16, kind="ExternalInput")
    xkv_d = nc.dram_tensor("xkv", [CP, N], bf16, kind="ExternalInput")
    out_d = nc.dram_tensor("out", [NC, N], f32, kind="ExternalOutput")

    with tile.TileContext(nc) as tc:
        with tc.tile_pool(name="persist", bufs=1) as pp:
            wall = pp.tile([CP, WALL], bf16)
            xq = pp.tile([CP, N], bf16)
            xkv = pp.tile([CP, N], bf16)
            nc.sync.dma_start(out=wall, in_=wall_d[:])
            nc.sync.dma_start(out=xkv, in_=xkv_d[:])
            nc.sync.dma_start(out=xq, in_=xq_d[:])
            wqT = wall[:, 0:C8]
            wkT = wall[:, C8 : 2 * C8]
            wvT = wall[:, 2 * C8 : 2 * C8 + NC]
            wcA = wall[:, 2 * C8 + NC : WALL]

            bufQ = pp.tile([C8, N], bf16)
            bufK = pp.tile([C8, N], bf16)
            VT1 = pp.tile([128, MB * CP], bf16)  # per m-block [V''^T | 1]
            Y0 = pp.tile([NC, N], f32)  # wc@x_q + bc
            bcw = pp.tile([1, NC], bf16)  # ones row for 1/l broadcast
            nc.vector.memset(VT1, 1.0)
            nc.vector.memset(bcw, 1.0)

            # ---- projections + Y0 (k/v first: they gate the main loop and
            # only need the xkv DMA) ----
            with tc.tile_pool(name="ps_setup", bufs=1, space="PSUM") as pss:
                for c in range(NCH):
                    sl = slice(NB * c, NB * (c + 1))
                    pk = pss.tile([C8, NB], f32, tag="pqk", bufs=2)
                    nc.tensor.matmul(pk, lhsT=wkT, rhs=xkv[:, sl], start=True, stop=True)
                    nc.vector.tensor_copy(bufK[:, sl], pk)
                for mb in range(MB):
                    pv = pss.tile([128, NC], f32, tag="pv", bufs=4)
                    nc.tensor.matmul(
                        pv,
                        lhsT=xkv[:, 128 * mb : 128 * (mb + 1)],
                        rhs=wvT,
                        start=True,
                        stop=True,
                    )
                    nc.vector.tensor_copy(VT1[:, CP * mb : CP * mb + NC], pv)
                for c in range(NCH):
                    sl = slice(NB * c, NB * (c + 1))
                    pq = pss.tile([C8, NB], f32, tag="pqk", bufs=2)
                    nc.tensor.matmul(pq, lhsT=wqT, rhs=xq[:, sl], start=True, stop=True)
                    nc.vector.tensor_copy(bufQ[:, sl], pq)
                    py0 = pss.tile([NC, NB], f32, tag="py0", bufs=2)
                    nc.tensor.matmul(py0, lhsT=wcA, rhs=xq[:, sl], start=True, stop=True)
                    nc.vector.tensor_copy(Y0[:, sl], py0)

            # ---- main loop: scores -> exp -> A@V, one flat software
            # pipeline across all 128 (n-chunk, m-chunk) pairs so the PE
            # never drains at n-chunk boundaries; epilogues trail by one
            # m-chunk ----
            with (
                tc.tile_pool(name="ps_main", bufs=1, space="PSUM") as psm,
                tc.tile_pool(name="epool", bufs=1) as ep,
            ):
                ps_os = {}

                def emit_epilogue(c):
                    nsl = slice(NB * c, NB * (c + 1))
                    oun = ep.tile([CP, NB], f32, tag="oun", bufs=2)
                    nc.vector.tensor_copy(oun, ps_os.pop(c))
                    rr = ep.tile([1, NB], bf16, tag="rr", bufs=2)
                    with nc.allow_low_precision(
                        reason="bf16 1/l adds ~0.2% noise on the attention term only"
                    ):
                        nc.vector.reciprocal(out=rr, in_=oun[NC : NC + 1, :])
                    R = psm.tile([NC, NB], f32, tag="R", bufs=1)
                    nc.tensor.matmul(R, lhsT=bcw, rhs=rr, start=True, stop=True)
                    yb = ep.tile([NC, NB], f32, tag="yb", bufs=2)
                    nc.vector.tensor_mul(out=yb, in0=oun[0:NC, :], in1=R)
                    nc.vector.tensor_add(out=yb, in0=yb, in1=Y0[:, nsl])
                    nc.sync.dma_start(out=out_d[:, nsl], in_=yb)

                pending = None  # (e_tile, c, mb0)
                for g in range(NCH * MCH + 1):
                    c, t = divmod(g, MCH)
                    if g < NCH * MCH:
                        nsl = slice(NB * c, NB * (c + 1))
                        if t == 0:
                            ps_os[c] = psm.tile(
                                [CP, NB], f32, tag="ps_o", bufs=2, name=f"ps_o{c}"
                            )
                        sc = psm.tile([128, 2 * NB], f32, tag=f"sc{g % 2}", bufs=1)
                        for s in range(2):
                            m0 = 128 * (2 * t + s)
                            nc.tensor.matmul(
                                sc[:, NB * s : NB * (s + 1)],
                                lhsT=bufK[:, m0 : m0 + 128],
                                rhs=bufQ[:, nsl],
                                start=True,
                                stop=True,
                            )
                    if pending is not None:
                        pe, pc, pmb0 = pending
                        for s in range(2):
                            mb = pmb0 + s
                            nc.tensor.matmul(
                                ps_os[pc],
                                lhsT=VT1[:, CP * mb : CP * (mb + 1)],
                                rhs=pe[:, NB * s : NB * (s + 1)],
                                start=(mb == 0),
                                stop=(mb == MB - 1),
                            )
                    if g < NCH * MCH:
                        e = ep.tile([128, 2 * NB], bf16, tag="e", bufs=3)
                        nc.scalar.activation(
                            out=e,
                            in_=sc,
                            func=mybir.ActivationFunctionType.Exp,
                        )
                        pending = (e, c, 2 * t)
                    # chunk c-? finished its stop-AV at iteration k*MCH;
                    # emit its epilogue one iteration later so the tiny
                    # R matmul never stalls the PE stream
                    if g >= MCH + 1 and (g - 1) % MCH == 0:
                        emit_epilogue((g - 1) // MCH - 1)
                emit_epilogue(NCH - 1)

    _split_multi_waits(nc)
    return nc


def _get_program():
    global _PROG
    if _PROG is None:
        _PROG = _build_program()
    return _PROG


TRACE = False
LAST_RESULT = None


def _to_bf16(a):
    import ml_dtypes

    return np.asarray(a, np.float32).astype(ml_dtypes.bfloat16)


def _pack_weights(wq, bq, wk, bk, wv, bv, gamma, wc, bc):
    g = float(np.asarray(gamma).reshape(-1)[0])
    wqT = np.zeros((CP, C8), np.float32)
    wqT[0:C, :] = wq.T
    wqT[C, :] = bq
    wkT = np.zeros((CP, C8), np.float32)
    wkT[0:C, :] = wk.T
    wkT[C, :] = bk
    # fold gamma*wc into the V projection
    w2 = g * (wc @ wv)  # [NC, C]
    b2 = g * (wc @ bv)  # [NC]
    wvT = np.zeros((CP, NC), np.float32)
    wvT[0:C, :] = w2.T
    wvT[C, :] = b2
    wcA = np.zeros((CP, NC), np.float32)
    wcA[0:C, :] = wc.T
    wcA[C, :] = bc
    return _to_bf16(np.concatenate([wqT, wkT, wvT, wcA], axis=1))


def _pack_x(x):
    buf = np.empty((CP, N), np.float32)
    buf[0:C, :] = x.reshape(C, N)
    buf[C, :] = 1.0
    return _to_bf16(buf)


def kernel(x1, x2, wq1, bq1, wk1, bk1, wv1, bv1, wq2, bq2, wk2, bk2,
           wv2, bv2, gamma1, gamma2, wc1, bc1, wc2, bc2):
    from concourse.bass_utils import run_bass_kernel_spmd

    global LAST_RESULT
    x1 = np.asarray(x1, np.float32)
    x2 = np.asarray(x2, np.float32)

    w1 = _pack_weights(
        np.asarray(wq1), np.asarray(bq1), np.asarray(wk2), np.asarray(bk2),
        np.asarray(wv2), np.asarray(bv2), np.asarray(gamma1),
        np.asarray(wc1), np.asarray(bc1),
    )
    w2 = _pack_weights(
        np.asarray(wq2), np.asarray(bq2), np.asarray(wk1), np.asarray(bk1),
        np.asarray(wv1), np.asarray(bv1), np.asarray(gamma2),
        np.asarray(wc2), np.asarray(bc2),
    )

    in_maps = []
    for d in range(2):
        xs_q, xs_kv = (x1, x2) if d == 0 else (x2, x1)
        wall = w1 if d == 0 else w2
        for b in range(B):
            in_maps.append(
                {
                    "xq": _pack_x(xs_q[b]),
                    "xkv": _pack_x(xs_kv[b]),
                    "wall": wall,
                }
            )

    nc = _get_program()
    res = run_bass_kernel_spmd(nc, in_maps, core_ids=list(range(8)), trace=TRACE)
    LAST_RESULT = res

    out1 = np.stack([res.results[b]["out"].reshape(C, 64, 64) for b in range(B)])
    out2 = np.stack([res.results[B + b]["out"].reshape(C, 64, 64) for b in range(B)])
    return out1.astype(np.float32), out2.astype(np.float32)


# revision 19
# speedup vs baseline: 1.1607x; 1.1607x over previous
"""Trainium2 Bass kernel for dual cross-attention (CotSR) block.

Problem (hardcoded shapes): B=4, C=96, H=W=64 -> N=4096, C8=12, NC=96.
  For each direction d and batch b:
    q = wq @ x_q + bq            [12, N]
    k = wk @ x_kv + bk           [12, N]
    v = wv @ x_kv + bv           [96, N]
    S = q^T k  (S[n, m])         softmax over m (unscaled)
    O = sum_m softmax(S)[n, m] * v[:, m]
    out = wc @ (x_q + gamma * O) + bc

Sharding: 8 independent (direction, batch) units -> 8 NeuronCores.

Per-core device pipeline (v2 — bf16 matmuls, fused epilogue):
  - the output conv is folded into the V projection host-side:
    W2 = gamma*wc@wv, b2 = gamma*wc@bv, so the A@V matmul directly
    produces the final attention contribution (pre-normalization);
    Y0 = wc@x_q + bc is computed once in the projection phase.
  - all big matmuls run in bf16 (1 PE cycle/row vs 4 for fp32):
    projections (K=97 incl. bias-through-ones-row), scores (K=12),
    A@V (K=128 per m-block, 97 rows incl. ones row for the softmax
    denominator).
  - scores computed transposed (S^T[m, n]) so the A@V matmul needs no
    transposes: per n-chunk of 512 query rows, m-chunks of 3/2 PSUM
    banks ping-pong; exp on ScalarE (ACT) straight out of PSUM into
    bf16 SBUF tiles; A@V software-pipelined one m-chunk behind.
  - per-chunk epilogue (software-pipelined one n-chunk behind): copy
    O'[97,512] to SBUF, reciprocal of row 96 (=denominator) on DVE,
    rank-1 fp32r matmul broadcasts 1/l to [96,512], then DVE
    multiply + add Y0 and DMA out.
"""

import numpy as np

B, C, C8, NC = 4, 96, 12, 96
N = 4096  # 64*64
CP = 97  # C + ones row
NCH = 8  # n chunks of 512
NB = 512
MB = 32  # m blocks of 128 per n-chunk
MCH = 16  # m-chunks of 2 blocks each; uniform so sc tags alternate globally

_PROG = None


def _split_multi_waits(nc):
    """Walrus in this container rejects >1 sync wait per instruction.
    Split extra waits onto same-engine NOPs inserted just before."""
    import concourse.mybir as mybir

    n_split = 0
    for bb in nc.main_func.blocks:
        insts = list(bb.instructions)
        if not any(i.sync_info and len(i.sync_info.on_wait) > 1 for i in insts):
            continue
        new = []
        for inst in insts:
            si = inst.sync_info
            if si is not None and len(si.on_wait) > 1:
                waits = list(si.on_wait)
                for w in waits[:-1]:
                    n_split += 1
                    new.append(
                        mybir.InstNoOp(
                            name=f"{inst.name}-wsplit{n_split}",
                            engine=inst.engine,
                            ins=[],
                            outs=[],
                            sync_info=mybir.SyncInfo(on_wait=[w], on_update=[]),
                        )
                    )
                inst.sync_info = mybir.SyncInfo(
                    on_wait=[waits[-1]], on_update=list(si.on_update)
                )
            new.append(inst)
        bb.instructions = new
    return n_split


def _build_program():
    import concourse.bass as bass
    import concourse.mybir as mybir
    import concourse.tile as tile

    f32 = mybir.dt.float32
    f32r = mybir.dt.float32r
    bf16 = mybir.dt.bfloat16
    nc = bass.Bass()

    # all weights packed in one tensor: tiny-row DMAs (24B/partition) cost
    # 20-30us each on this part; one 432B-row DMA costs ~1us
    WALL = 2 * C8 + 2 * NC  # 216 cols: [wqT | wkT | wvT | wcA]
    wall_d = nc.dram_tensor("wall", [CP, WALL], bf16, kind="ExternalInput")
    xq_d = nc.dram_tensor("xq", [CP, N], bf16, kind="ExternalInput")
    xkv_d = nc.dram_tensor("xkv", [CP, N], bf16, kind="ExternalInput")
    out_d = nc.dram_tensor("out", [NC, N], f32, kind="ExternalOutput")

    with tile.TileContext(nc) as tc:
        with tc.tile_pool(name="persist", bufs=1) as pp:
            wall = pp.tile([CP, WALL], bf16)
            xq = pp.tile([CP, N], bf16)
            xkv = pp.tile([CP, N], bf16)
            # each HW DMA queue sustains only ~25 GB/s on this part; split
            # the two 776KB input transfers across the SP and ACT queues
            # (xkv first: k/v projections gate the main loop)
            HP = 49
            nc.sync.dma_start(out=wall, in_=wall_d[:])
            nc.sync.dma_start(out=xkv[0:HP, :], in_=xkv_d[0:HP, :])
            nc.scalar.dma_start(out=xkv[HP:CP, :], in_=xkv_d[HP:CP, :])
            nc.sync.dma_start(out=xq[0:HP, :], in_=xq_d[0:HP, :])
            nc.scalar.dma_start(out=xq[HP:CP, :], in_=xq_d[HP:CP, :])
            wqT = wall[:, 0:C8]
            wkT = wall[:, C8 : 2 * C8]
            wvT = wall[:, 2 * C8 : 2 * C8 + NC]
            wcA = wall[:, 2 * C8 + NC : WALL]

            bufQ = pp.tile([C8, N], bf16)
            bufK = pp.tile([C8, N], bf16)
            VT1 = pp.tile([128, MB * CP], bf16)  # per m-block [V''^T | 1]
            Y0 = pp.tile([NC, N], f32)  # wc@x_q + bc
            bcw = pp.tile([1, NC], bf16)  # ones row for 1/l broadcast
            nc.vector.memset(VT1, 1.0)
            nc.vector.memset(bcw, 1.0)

            # ---- projections + Y0 (k/v first: they gate the main loop and
            # only need the xkv DMA) ----
            with tc.tile_pool(name="ps_setup", bufs=1, space="PSUM") as pss:
                for c in range(NCH):
                    sl = slice(NB * c, NB * (c + 1))
                    pk = pss.tile([C8, NB], f32, tag="pqk", bufs=2)
                    nc.tensor.matmul(pk, lhsT=wkT, rhs=xkv[:, sl], start=True, stop=True)
                    nc.vector.tensor_copy(bufK[:, sl], pk)
                for mb in range(MB):
                    pv = pss.tile([128, NC], f32, tag="pv", bufs=4)
                    nc.tensor.matmul(
                        pv,
                        lhsT=xkv[:, 128 * mb : 128 * (mb + 1)],
                        rhs=wvT,
                        start=True,
                        stop=True,
                    )
                    nc.vector.tensor_copy(VT1[:, CP * mb : CP * mb + NC], pv)
                for c in range(NCH):
                    sl = slice(NB * c, NB * (c + 1))
                    pq = pss.tile([C8, NB], f32, tag="pqk", bufs=2)
                    nc.tensor.matmul(pq, lhsT=wqT, rhs=xq[:, sl], start=True, stop=True)
                    nc.vector.tensor_copy(bufQ[:, sl], pq)
                    py0 = pss.tile([NC, NB], f32, tag="py0", bufs=2)
                    nc.tensor.matmul(py0, lhsT=wcA, rhs=xq[:, sl], start=True, stop=True)
                    nc.vector.tensor_copy(Y0[:, sl], py0)

            # ---- main loop: scores -> exp -> A@V, one flat software
            # pipeline across all 128 (n-chunk, m-chunk) pairs so the PE
            # never drains at n-chunk boundaries; epilogues trail by one
            # m-chunk ----
            with (
                tc.tile_pool(name="ps_main", bufs=1, space="PSUM") as psm,
                tc.tile_pool(name="epool", bufs=1) as ep,
            ):
                ps_os = {}

                def emit_epilogue(c):
                    nsl = slice(NB * c, NB * (c + 1))
                    oun = ep.tile([CP, NB], f32, tag="oun", bufs=2)
                    nc.vector.tensor_copy(oun, ps_os.pop(c))
                    rr = ep.tile([1, NB], bf16, tag="rr", bufs=2)
                    with nc.allow_low_precision(
                        reason="bf16 1/l adds ~0.2% noise on the attention term only"
                    ):
                        nc.vector.reciprocal(out=rr, in_=oun[NC : NC + 1, :])
                    R = psm.tile([NC, NB], f32, tag="R", bufs=1)
                    nc.tensor.matmul(R, lhsT=bcw, rhs=rr, start=True, stop=True)
                    yb = ep.tile([NC, NB], f32, tag="yb", bufs=2)
                    nc.vector.tensor_mul(out=yb, in0=oun[0:NC, :], in1=R)
                    nc.vector.tensor_add(out=yb, in0=yb, in1=Y0[:, nsl])
                    # alternate output DMAs across the two HW queues
                    eng = nc.sync if c % 2 == 0 else nc.scalar
                    eng.dma_start(out=out_d[:, nsl], in_=yb)

                pending = None  # (e_tile, c, mb0)
                for g in range(NCH * MCH + 1):
                    c, t = divmod(g, MCH)
                    if g < NCH * MCH:
                        nsl = slice(NB * c, NB * (c + 1))
                        if t == 0:
                            ps_os[c] = psm.tile(
                                [CP, NB], f32, tag="ps_o", bufs=2, name=f"ps_o{c}"
                            )
                        sc = psm.tile([128, 2 * NB], f32, tag=f"sc{g % 2}", bufs=1)
                        for s in range(2):
                            m0 = 128 * (2 * t + s)
                            nc.tensor.matmul(
                                sc[:, NB * s : NB * (s + 1)],
                                lhsT=bufK[:, m0 : m0 + 128],
                                rhs=bufQ[:, nsl],
                                start=True,
                                stop=True,
                            )
                    if pending is not None:
                        pe, pc, pmb0 = pending
                        for s in range(2):
                            mb = pmb0 + s
                            nc.tensor.matmul(
                                ps_os[pc],
                                lhsT=VT1[:, CP * mb : CP * (mb + 1)],
                                rhs=pe[:, NB * s : NB * (s + 1)],
                                start=(mb == 0),
                                stop=(mb == MB - 1),
                            )
                    if g < NCH * MCH:
                        e = ep.tile([128, 2 * NB], bf16, tag="e", bufs=3)
                        nc.scalar.activation(
                            out=e,
                            in_=sc,
                            func=mybir.ActivationFunctionType.Exp,
                        )
                        pending = (e, c, 2 * t)
                    # chunk c-? finished its stop-AV at iteration k*MCH;
                    # emit its epilogue one iteration later so the tiny
                    # R matmul never stalls the PE stream
                    if g >= MCH + 1 and (g - 1) % MCH == 0:
                        emit_epilogue((g - 1) // MCH - 1)
                emit_epilogue(NCH - 1)

    _split_multi_waits(nc)
    return nc


def _get_program():
    global _PROG
    if _PROG is None:
        _PROG = _build_program()
    return _PROG


TRACE = False
LAST_RESULT = None


def _to_bf16(a):
    import ml_dtypes

    return np.asarray(a, np.float32).astype(ml_dtypes.bfloat16)


def _pack_weights(wq, bq, wk, bk, wv, bv, gamma, wc, bc):
    g = float(np.asarray(gamma).reshape(-1)[0])
    wqT = np.zeros((CP, C8), np.float32)
    wqT[0:C, :] = wq.T
    wqT[C, :] = bq
    wkT = np.zeros((CP, C8), np.float32)
    wkT[0:C, :] = wk.T
    wkT[C, :] = bk
    # fold gamma*wc into the V projection
    w2 = g * (wc @ wv)  # [NC, C]
    b2 = g * (wc @ bv)  # [NC]
    wvT = np.zeros((CP, NC), np.float32)
    wvT[0:C, :] = w2.T
    wvT[C, :] = b2
    wcA = np.zeros((CP, NC), np.float32)
    wcA[0:C, :] = wc.T
    wcA[C, :] = bc
    return _to_bf16(np.concatenate([wqT, wkT, wvT, wcA], axis=1))


def _pack_x(x):
    buf = np.empty((CP, N), np.float32)
    buf[0:C, :] = x.reshape(C, N)
    buf[C, :] = 1.0
    return _to_bf16(buf)


def kernel(x1, x2, wq1, bq1, wk1, bk1, wv1, bv1, wq2, bq2, wk2, bk2,
           wv2, bv2, gamma1, gamma2, wc1, bc1, wc2, bc2):
    from concourse.bass_utils import run_bass_kernel_spmd

    global LAST_RESULT
    x1 = np.asarray(x1, np.float32)
    x2 = np.asarray(x2, np.float32)

    w1 = _pack_weights(
        np.asarray(wq1), np.asarray(bq1), np.asarray(wk2), np.asarray(bk2),
        np.asarray(wv2), np.asarray(bv2), np.asarray(gamma1),
        np.asarray(wc1), np.asarray(bc1),
    )
    w2 = _pack_weights(
        np.asarray(wq2), np.asarray(bq2), np.asarray(wk1), np.asarray(bk1),
        np.asarray(wv1), np.asarray(bv1), np.asarray(gamma2),
        np.asarray(wc2), np.asarray(bc2),
    )

    in_maps = []
    for d in range(2):
        xs_q, xs_kv = (x1, x2) if d == 0 else (x2, x1)
        wall = w1 if d == 0 else w2
        for b in range(B):
            in_maps.append(
                {
                    "xq": _pack_x(xs_q[b]),
                    "xkv": _pack_x(xs_kv[b]),
                    "wall": wall,
                }
            )

    nc = _get_program()
    res = run_bass_kernel_spmd(nc, in_maps, core_ids=list(range(8)), trace=TRACE)
    LAST_RESULT = res

    out1 = np.stack([res.results[b]["out"].reshape(C, 64, 64) for b in range(B)])
    out2 = np.stack([res.results[B + b]["out"].reshape(C, 64, 64) for b in range(B)])
    return out1.astype(np.float32), out2.astype(np.float32)
